# revision 41
# baseline (speedup 1.0000x reference)
"""Trainium2 Bass kernel for nn_MultiHeadAttention_377957122345.

B=16, T=512, C=1024, H=16, D=64.  Data-parallel over batch: each of the
8 NeuronCores computes attention for 2 sequences; no collectives.

Per-core device program (SPMD, identical on all cores):
  - all large inputs are pre-cast to bf16 on host and DMAed straight
    into SBUF: xT [C, NT] (c_in on partitions), W^T [c_in, c_out] for
    the four projections, and the causal-masked multiplicative bias
    exp(rel_pos_bias + mask), host-packed per head as the transposed
    lower-triangle blocks [128, 1280].
  - all matmuls in bf16 with fp32 PSUM accumulation.
  - Q/K projections produce q^T/k^T (head pair stacked on the 128
    partitions: even head at rows 0-63, odd head at rows 64-127), so
    the two heads' S^T = k_j^T q matmuls run row-tiled (tile_position
    (0,0)/(64,0)) concurrently on the PE array.
  - scores are computed transposed (S^T[s, t]); exp on ScalarE, bias
    multiply on DVE (bf16 2x mode); causal block skipping throughout.
  - AV: V is kept ones-augmented ([s, 128] per head: V in cols 0-63,
    ones in cols 64-127) so one accumulating matmul per key block j
    emits the attention output (rows 0-63) AND the softmax denominator
    replicated across rows 64-127 of the same PSUM tile — matmul time
    depends only on streamed rhs columns, so the replication is free.
  - normalization: reciprocal_approx_fast on the replicated denominator
    rows (64 parallel lanes) and one fused DVE multiply writing the
    normalized o^T straight into the out-projection layout.
  - y is produced in bf16 and upcast on host.
"""

import numpy as np

B, T, C, H = 16, 512, 1024, 16
D = C // H  # 64
N_CORES = 8
B_LOC = B // N_CORES  # 2 sequences per core
NT = B_LOC * T  # 1024 tokens per core
P = 128
KO = C // P  # 8 contraction subtiles
TB = T // P  # 4 query blocks per sequence
NCH = C // 512  # 2 output chunks of 512
NEG = -1e30
WJS = [T - j * P for j in range(TB)]  # 512, 384, 256, 128
JOFF = [0]
for _w in WJS[:-1]:
    JOFF.append(JOFF[-1] + _w)
BW = JOFF[-1] + WJS[-1]  # 1280 packed bias columns per head

_CACHE = {}

import contextlib


def _nullcm():
    return contextlib.nullcontext()


def _split_big_waits(nc, mybir, limit=1):
    # This walrus build rejects instructions whose sync_info.on_wait
    # exceeds its slot count (the Tile end-of-kernel Drain trips it).
    # Move excess waits onto dedicated same-engine NOPs placed directly
    # before the instruction; the engine stalls on those first, so the
    # semantics are unchanged.
    f = nc.m.functions[0]
    for bb in f.blocks:
        out = []
        changed = False
        for inst in bb.instructions:
            si = getattr(inst, "sync_info", None)
            waits = list(si.on_wait) if si is not None and si.on_wait else []
            if len(waits) > limit:
                changed = True
                head, tail = waits[:-limit], waits[-limit:]
                for k in range(0, len(head), limit):
                    out.append(
                        mybir.InstNoOp(
                            name=f"I-{nc.next_id()}",
                            sync_info=mybir.SyncInfo(
                                on_wait=head[k : k + limit], on_update=[]
                            ),
                            bass_nofuse=True,
                            engine=inst.engine,
                        )
                    )
                si.on_wait = tail
            out.append(inst)
        if changed:
            bb.instructions = out
    return nc


def _act_reciprocal(nc, mybir, out, in_):
    """InstActivation(func=Reciprocal) on ScalarE.  bass's activation()
    refuses Reciprocal for accuracy reasons; here the operands are softmax
    denominators (well-conditioned, positive) and the tolerance budget is
    2e-2, so the table-based ACT reciprocal is fine — validated against
    the fp32 reference."""
    eng = nc.scalar
    imm = lambda v: mybir.ImmediateValue(dtype=mybir.dt.float32, value=v)
    return eng.add_instruction(
        mybir.InstActivation(
            name=nc.get_next_instruction_name(),
            func=mybir.ActivationFunctionType.Reciprocal,
            ins=[eng.lower_ap(in_), imm(0.0), imm(1.0), imm(0.0)],
            outs=[eng.lower_ap(out)],
        )
    )


def build_program(split_waits=True, reps=1, parts=("proj", "attn", "oproj"), abl=()):
    import concourse.bass as bass
    import concourse.mybir as mybir
    import concourse.tile as tile

    fp32 = mybir.dt.float32
    bf16 = mybir.dt.bfloat16
    Act = mybir.ActivationFunctionType

    nc = bass.Bass()
    xT = nc.dram_tensor("xT", [C, NT], bf16, kind="ExternalInput")
    wqT = nc.dram_tensor("wqT", [C, C], bf16, kind="ExternalInput")
    wkT = nc.dram_tensor("wkT", [C, C], bf16, kind="ExternalInput")
    wvT = nc.dram_tensor("wvT", [C, C], bf16, kind="ExternalInput")
    woT = nc.dram_tensor("woT", [C, C], bf16, kind="ExternalInput")
    bqd = nc.dram_tensor("bq", [C], fp32, kind="ExternalInput")
    bkd = nc.dram_tensor("bk", [C], fp32, kind="ExternalInput")
    bvd = nc.dram_tensor("bv", [C], fp32, kind="ExternalInput")
    bod = nc.dram_tensor("bo", [C], fp32, kind="ExternalInput")
    biasm = nc.dram_tensor("biasm", [H // 2, P, 2, BW], bf16, kind="ExternalInput")
    y = nc.dram_tensor("y", [NT, C], bf16, kind="ExternalOutput")

    with tile.TileContext(nc) as tc, \
         tc.tile_pool(name="consts", bufs=1) as consts, \
         tc.tile_pool(name="persist", bufs=1) as persist, \
         tc.tile_pool(name="biasp", bufs=2) as biasp, \
         tc.tile_pool(name="soft", bufs=6) as soft, \
         tc.tile_pool(name="stagep", bufs=1) as stagep, \
         tc.tile_pool(name="ypool", bufs=3) as ypool, \
         tc.tile_pool(name="psA", bufs=2, space="PSUM") as psA, \
         tc.tile_pool(name="psS", bufs=2, space="PSUM") as psS, \
         tc.tile_pool(name="psO", bufs=2, space="PSUM") as psO:

        # ----- persistent tiles + loop-invariant setup (ones columns) ----
        xT_bf = persist.tile([P, KO, NT], bf16, name="xT_bf")
        wq_bf = persist.tile([P, KO, C], bf16, name="wq_bf")
        wk_bf = persist.tile([P, KO, C], bf16, name="wk_bf")
        wv_bf = persist.tile([P, KO, C], bf16, name="wv_bf")
        wo_bf = persist.tile([P, KO, C], bf16, name="wo_bf")
        qT_bf = persist.tile([P, KO, NT], bf16, name="qT_bf")
        kT_bf = persist.tile([P, KO, NT], bf16, name="kT_bf")
        # vaug: per head [V_h | ones] (128 weight cols), so the AV matmul
        # emits o on PSUM rows 0-63 and the softmax denominator
        # replicated on rows 64-127 at no extra streaming cost.
        vaug = persist.tile([P, NT // P, H, 2 * D], bf16, name="vaug")
        oT_bf = persist.tile([P, KO, NT], bf16, name="oT_bf")
        for to in range(NT // P):
            nc.vector.memset(vaug[:, to, :, D:], 1.0)
        if "attn" not in parts:
            nc.vector.memset(oT_bf[:], 0.0)

        loop_cm = tc.For_i(0, reps, 1) if reps > 1 else _nullcm()
        with loop_cm:
            # ----- constants -----
            bq_sb = consts.tile([P, KO], fp32, name="bq_sb")
            nc.sync.dma_start(out=bq_sb, in_=bqd.rearrange("(o p) -> p o", p=P))
            bk_sb = consts.tile([P, KO], fp32, name="bk_sb")
            nc.sync.dma_start(out=bk_sb, in_=bkd.rearrange("(o p) -> p o", p=P))
            bv_sb = consts.tile([P, C], fp32, name="bv_sb")
            bv_ap = bvd[:]
            nc.sync.dma_start(
                out=bv_sb,
                in_=bass.AP(
                    tensor=bv_ap.tensor, offset=bv_ap.offset, ap=[[0, P], [1, C]]
                ),
            )
            bo_sb = consts.tile([P, C], fp32, name="bo_sb")
            bo_ap = bod[:]
            nc.sync.dma_start(
                out=bo_sb,
                in_=bass.AP(
                    tensor=bo_ap.tensor, offset=bo_ap.offset, ap=[[0, P], [1, C]]
                ),
            )

            # ----- load bf16 inputs straight into SBUF -----
            # xT per (ko, half) with the nch=0 halves first and wq in
            # mo-pair chunks (512B rows) so the first Q-proj matmuls can
            # start after ~1.3MB of DMA; wk/wv/wo as single large DMAs.
            for ko in range(KO):
                nc.sync.dma_start(
                    out=xT_bf[:, ko, 0:512],
                    in_=xT[ko * P : (ko + 1) * P, 0:512],
                )
            wq_ap = wqT[:]
            for mop in range(4):
                nc.sync.dma_start(
                    out=wq_bf[:, :, mop * 256 : (mop + 1) * 256],
                    in_=bass.AP(
                        tensor=wq_ap.tensor,
                        offset=wq_ap.offset + mop * 256,
                        ap=[[C, P], [P * C, KO], [1, 256]],
                    ),
                )
            for ko in range(KO):
                nc.sync.dma_start(
                    out=xT_bf[:, ko, 512:1024],
                    in_=xT[ko * P : (ko + 1) * P, 512:1024],
                )
            nc.sync.dma_start(out=wk_bf, in_=wkT.rearrange("(o p) c -> p o c", p=P))
            nc.sync.dma_start(out=wv_bf, in_=wvT.rearrange("(o p) c -> p o c", p=P))
            if "oproj" in parts:
                nc.sync.dma_start(
                    out=wo_bf, in_=woT.rearrange("(o p) c -> p o c", p=P)
                )

            # ----- Q/K projections -> transposed layout [c_out on parts, t]
            # q is pre-scaled by 1/sqrt(D) (bq comes pre-scaled from host).
            for w_bf, out_bf, b_sb, scl in (
                (wq_bf, qT_bf, bq_sb, 1.0 / np.sqrt(D)),
                (wk_bf, kT_bf, bk_sb, 1.0),
            ):
                for nch in range(NCH):
                    for mo in range(KO):
                        ps = psA.tile([P, 512], fp32, tag="psA")
                        for ko in range(KO):
                            nc.tensor.matmul(
                                ps,
                                lhsT=w_bf[:, ko, mo * P : (mo + 1) * P],
                                rhs=xT_bf[:, ko, nch * 512 : (nch + 1) * 512],
                                start=(ko == 0),
                                stop=(ko == KO - 1),
                            )
                        nc.scalar.activation(
                            out=out_bf[:, mo, nch * 512 : (nch + 1) * 512],
                            in_=ps,
                            func=Act.Identity,
                            bias=b_sb[:, mo : mo + 1],
                            scale=scl,
                        )

            # ----- V projection / out projection tile emitters -----
            def emit_vproj(to, nch):
                ps = psA.tile([P, 512], fp32, tag="psA")
                for ko in range(KO):
                    nc.tensor.matmul(
                        ps,
                        lhsT=xT_bf[:, ko, to * P : (to + 1) * P],
                        rhs=wv_bf[:, ko, nch * 512 : (nch + 1) * 512],
                        start=(ko == 0),
                        stop=(ko == KO - 1),
                    )
                nc.any.tensor_add(
                    out=vaug[:, to, nch * 8 : (nch + 1) * 8, 0:D],
                    in0=ps,
                    in1=bv_sb[:, nch * 512 : (nch + 1) * 512],
                )

            def emit_oproj(to, nch):
                ps = psA.tile([P, 512], fp32, tag="psA")
                for co in range(KO):
                    nc.tensor.matmul(
                        ps,
                        lhsT=oT_bf[:, co, to * P : (to + 1) * P],
                        rhs=wo_bf[:, co, nch * 512 : (nch + 1) * 512],
                        start=(co == 0),
                        stop=(co == KO - 1),
                    )
                ysb = ypool.tile([P, 512], bf16, tag="y")
                nc.any.tensor_add(
                    out=ysb, in0=ps, in1=bo_sb[:, nch * 512 : (nch + 1) * 512]
                )
                nc.sync.dma_start(
                    out=y[to * P : (to + 1) * P, nch * 512 : (nch + 1) * 512],
                    in_=ysb,
                )

            # ----- attention group: head pair (2hp, 2hp+1) of sequence b.
            # Both heads' S^T matmuls are row-tiled on the PE array; their
            # exp / bias-multiply run as single paired ops over a 2-bank
            # PSUM tile.
            def attn_group(b, hp, stage, fill_fn=None):
                bt = biasp.tile([P, 2, BW], bf16, tag="bias")
                nc.sync.dma_start(out=bt, in_=biasm[hp])
                PTs = []
                for j in range(TB):
                    wj = WJS[j]
                    psSt = psS.tile([P, 2, 512], fp32, tag="psS")
                    for hh in range(2):
                        po = hh * D
                        nc.tensor.matmul(
                            psSt[:, hh, :wj],
                            lhsT=kT_bf[
                                po : po + D,
                                hp,
                                b * T + j * P : b * T + (j + 1) * P,
                            ],
                            rhs=qT_bf[po : po + D, hp, b * T + j * P : (b + 1) * T],
                            start=True,
                            stop=True,
                        )
                    PT = soft.tile([P, 2, 512], bf16, tag="PT")
                    nc.scalar.activation(
                        out=PT[:, :, :wj], in_=psSt[:, :, :wj], func=Act.Exp
                    )
                    if "no_bmul" not in abl:
                        nc.vector.tensor_mul(
                            out=PT[:, :, :wj],
                            in0=PT[:, :, :wj],
                            in1=bt[:, :, JOFF[j] : JOFF[j] + wj],
                        )
                    PTs.append(PT)
                # dense filler work (a projection tile) keeps the in-order
                # PE queue busy while the exp/bias-mul pipeline catches up
                # before the AV matmuls.
                if fill_fn is not None:
                    fill_fn()
                for hh in range(2):
                    h = 2 * hp + hh
                    pst = psO.tile([P, 512], fp32, tag="psO")
                    for j in range(TB):
                        nc.tensor.matmul(
                            pst[:, j * P :],
                            lhsT=vaug[:, b * TB + j, h, :],
                            rhs=PTs[j][:, hh, : WJS[j]],
                            start=(j == 0),
                            stop=(j == TB - 1),
                            skip_group_check=True,
                        )
                    # unnormalized o goes straight to its oT home; the
                    # replicated denominators land at the head's parity
                    # partitions of the stage tile.  Reciprocals for the
                    # whole sequence then run as ONE ACT op (a single
                    # table-set switch instead of one per head) and the
                    # normalize-multiplies run in place on oT, all APs
                    # partition-aligned.
                    po = hh * D
                    nc.any.tensor_copy(
                        out=oT_bf[po : po + D, hp, b * T : (b + 1) * T],
                        in_=pst[0:D, :],
                    )
                    nc.any.tensor_copy(
                        out=stage[po : po + D, hp, :], in_=pst[D : 2 * D, :]
                    )

            # ----- per-sequence epilogue: one batched reciprocal over all
            # 16 heads' replicated denominators (single ACT table-set
            # switch), then the fused normalize-multiplies into oT.
            def finish_seq(b, stage):
                _act_reciprocal(nc, mybir, stage[:, :, :], stage[:, :, :])
                for h in range(H):
                    hp, hh = h // 2, h % 2
                    po = hh * D
                    sl = oT_bf[po : po + D, hp, b * T : (b + 1) * T]
                    nc.vector.tensor_mul(
                        out=sl, in0=sl, in1=stage[po : po + D, hp, :]
                    )

            # ----- phase schedule: V-proj for seq 0, then attention(seq 0)
            # with V-proj(seq 1) tiles interleaved to keep the PE dense,
            # then attention(seq 1) with out-proj(seq 0) interleaved, then
            # out-proj(seq 1).
            have_attn = "attn" in parts
            have_oproj = "oproj" in parts
            for to in range(TB):
                for nch in range(NCH):
                    emit_vproj(to, nch)
            if have_attn:
                stage0 = stagep.tile([P, H // 2, T], bf16, tag="stage")
                for hp in range(H // 2):
                    attn_group(
                        0, hp, stage0,
                        fill_fn=lambda hp=hp: emit_vproj(TB + hp // 2, hp % 2),
                    )
                finish_seq(0, stage0)
                stage1 = stagep.tile([P, H // 2, T], bf16, tag="stage")
                for hp in range(H // 2):
                    fill = None
                    if have_oproj:
                        fill = lambda hp=hp: emit_oproj(hp // 2, hp % 2)
                    attn_group(1, hp, stage1, fill_fn=fill)
                finish_seq(1, stage1)
                if have_oproj:
                    for to in range(TB, NT // P):
                        for nch in range(NCH):
                            emit_oproj(to, nch)
            else:
                for hp in range(H // 2):
                    emit_vproj(TB + hp // 2, hp % 2)
                if have_oproj:
                    for to in range(NT // P):
                        for nch in range(NCH):
                            emit_oproj(to, nch)
            if not have_oproj:
                # keep the output defined for ablation runs
                zsb = ypool.tile([P, 512], bf16, tag="y")
                nc.any.memset(zsb[:], 0.0)
                nc.sync.dma_start(out=y[0:P, 0:512], in_=zsb)

    if split_waits:
        _split_big_waits(nc, mybir, limit=1)
    return nc


def make_in_maps(inputs):
    import ml_dtypes

    bf = ml_dtypes.bfloat16
    x = np.asarray(inputs["x"], dtype=np.float32)
    wT = {
        k: np.ascontiguousarray(
            np.asarray(inputs[f"W{k}"], dtype=np.float32).T.astype(bf)
        )
        for k in "qkvo"
    }
    bq = np.asarray(inputs["bq"], dtype=np.float32) * np.float32(1.0 / np.sqrt(D))
    bk = np.asarray(inputs["bk"], dtype=np.float32)
    bv = np.asarray(inputs["bv"], dtype=np.float32)
    bo = np.asarray(inputs["bo"], dtype=np.float32)

    bm = np.asarray(inputs["rel_pos_bias"], dtype=np.float32)[:, :T, :T].copy()
    iu = np.triu_indices(T, 1)
    bm[:, iu[0], iu[1]] = NEG
    # multiplicative form: exp(S+bias) = exp(S) * exp(bias); causal mask
    # becomes an exact multiplicative zero.  Transposed to [h, s, t] and
    # packed per head as the 4 causal blocks [s=jP..(j+1)P, t=jP..T]
    # side by side -> [H, 128, 1280].
    bmT = np.exp(bm.transpose(0, 2, 1))
    packed = np.zeros((H, P, BW), dtype=np.float32)
    for j in range(TB):
        packed[:, :, JOFF[j] : JOFF[j] + WJS[j]] = bmT[
            :, j * P : (j + 1) * P, j * P :
        ]
    # head pair (2hp, 2hp+1) packed side by side -> [H/2, 128, 2, BW]
    packed = packed.reshape(H // 2, 2, P, BW).transpose(0, 2, 1, 3)
    packed = np.ascontiguousarray(packed.astype(bf))

    xT_all = x.reshape(N_CORES, NT, C).transpose(0, 2, 1)
    in_maps = []
    for c in range(N_CORES):
        in_maps.append(
            {
                "xT": np.ascontiguousarray(xT_all[c].astype(bf)),
                "wqT": wT["q"],
                "wkT": wT["k"],
                "wvT": wT["v"],
                "woT": wT["o"],
                "bq": bq,
                "bk": bk,
                "bv": bv,
                "bo": bo,
                "biasm": packed,
            }
        )
    return in_maps


def build_jitted(nc, n_cores=N_CORES):
    """Build a persistent jitted shard_map executable for `nc` (the
    multi-core path of bass2jax.run_bass_via_pjrt, kept resident so repeat
    kernel() calls skip retracing)."""
    import jax
    from jax.experimental.shard_map import shard_map
    from jax.sharding import Mesh, NamedSharding, PartitionSpec

    from concourse import mybir
    from concourse.bass2jax import (
        _bass_exec_p,
        install_neuronx_cc_hook,
        partition_id_tensor,
    )

    install_neuronx_cc_hook()
    partition_name = nc.partition_id_tensor.name if nc.partition_id_tensor else None

    in_names, out_names, out_avals, zero_outs = [], [], [], []
    for alloc in nc.m.functions[0].allocations:
        if not isinstance(alloc, mybir.MemoryLocationSet):
            continue
        name = alloc.memorylocations[0].name
        if alloc.kind == "ExternalInput":
            if name != partition_name:
                in_names.append(name)
        elif alloc.kind == "ExternalOutput":
            out_names.append(name)
            shape = tuple(alloc.tensor_shape)
            dtype = mybir.dt.np(alloc.dtype)
            out_avals.append(jax.core.ShapedArray(shape, dtype))
            zero_outs.append(np.zeros(shape, dtype))
    n_params = len(in_names)
    n_outs = len(out_avals)
    all_in_names = list(in_names) + list(out_names)
    if partition_name is not None:
        all_in_names.append(partition_name)
    donate = tuple(range(n_params, n_params + n_outs))

    def _body(*args):
        operands = list(args)
        if partition_name is not None:
            operands.append(partition_id_tensor())
        outs = _bass_exec_p.bind(
            *operands,
            out_avals=tuple(out_avals),
            in_names=tuple(all_in_names),
            out_names=tuple(out_names),
            lowering_input_output_aliases=(),
            sim_require_finite=True,
            sim_require_nnan=True,
            nc=nc,
        )
        return tuple(outs)

    devices = jax.devices()[:n_cores]
    mesh = Mesh(np.asarray(devices), ("core",))
    in_specs = (PartitionSpec("core"),) * (n_params + n_outs)
    out_specs = (PartitionSpec("core"),) * n_outs
    jitted = jax.jit(
        shard_map(_body, mesh=mesh, in_specs=in_specs, out_specs=out_specs,
                  check_rep=False),
        donate_argnums=donate,
        keep_unused=True,
    )
    sharding = NamedSharding(mesh, PartitionSpec("core"))
    return jitted, in_names, out_names, out_avals, zero_outs, sharding


def get_runner():
    """Build the program + executable once; return in_maps -> per-core
    output dicts."""
    if "runner" in _CACHE:
        return _CACHE["runner"]
    import jax

    nc = build_program()
    jitted, in_names, out_names, out_avals, zero_outs, sharding = build_jitted(nc)
    n_cores = N_CORES

    def runner(in_maps):
        concat_in = [
            jax.device_put(
                np.concatenate(
                    [np.asarray(in_maps[c][nm]) for c in range(n_cores)], axis=0
                ),
                sharding,
            )
            for nm in in_names
        ]
        zeros = [
            jax.device_put(
                np.zeros((n_cores * z.shape[0], *z.shape[1:]), z.dtype), sharding
            )
            for z in zero_outs
        ]
        out_arrs = jitted(*concat_in, *zeros)
        return [
            {
                nm: np.asarray(out_arrs[i]).reshape(n_cores, *out_avals[i].shape)[c]
                for i, nm in enumerate(out_names)
            }
            for c in range(n_cores)
        ]

    _CACHE["runner"] = runner
    _CACHE["nc"] = nc
    return runner


def kernel(**inputs) -> np.ndarray:
    runner = get_runner()
    in_maps = make_in_maps(inputs)
    results = runner(in_maps)
    out = np.concatenate(
        [
            results[c]["y"].astype(np.float32).reshape(B_LOC, T, C)
            for c in range(N_CORES)
        ],
        axis=0,
    )
    return out
